# revision 6
# baseline (speedup 1.0000x reference)
"""Trainium2 Bass kernel for CollapsePreventionLoss.

reference:
    atoms = coordinates.reshape(B, N, 3)           # B=64, N=1024
    dist  = sqrt(pairwise_dist_sq + 1e-8)
    loss  = sum_{i<j} relu(2.9 - dist)^2 / B

Strategy (8 NeuronCores, data parallel over batch, 8 batches/core):
  dist_sq[i,j] = s_i + s_j - 2 a_i.a_j  as ONE K=18 bf16 matmul per chunk:
  every product is exact in fp32 (bf16 hi/lo split of each coordinate,
  3-way bf16 split of the squared norms), so dist_sq is the exact pair
  distance of slightly-perturbed atoms, plus an EPS_GUARD that keeps it
  positive (sqrt(neg) = NaN on the ACT engine).

  v3 layout:
  - PE row-group tiling: K=18 <= 32, so 4 independent matmuls run
    concurrently in the 4 32-row groups of the PE array
    (tile_position=(32q, 0), q = row_block % 4).  lhs/rhs live at SBUF
    partition base 32q (host replicates rhs into 4 partition groups).
  - 3 PSUM tiles of 1536 f32 (3 banks) per batch; matmul chunks <= 512
    never cross a bank boundary.  Tile 0 holds the eight 128-wide
    diagonal blocks (computed unmasked; host removes diag and halves
    by symmetry), tiles carry each row-block's off-diag columns.
  - ACT: one Sqrt per PSUM tile -> d bf16 in SBUF (the only ACT work).
  - DVE: t = min(d - 2.9, 0) fp16 (tensor_scalar, 4x mode),
         sq = t*t fp16 (tensor_tensor, 2x mode),
         accumulate sq (tensor_scalar mult-1/add-0 with accum_out, 4x),
         split diag-region / off-diag accumulators.
  Host sums the per-row partials in fp64 and applies the exactly
  emulated diagonal-element correction.
"""

import sys

for _p in ("/opt/trn_rl_repo",):
    if _p not in sys.path:
        sys.path.insert(0, _p)

import numpy as np

import concourse.bacc as bacc
import concourse.tile as tile
from concourse import mybir
from concourse.bass_utils import run_bass_kernel_spmd

B = 64
N = 1024
NCORES = 8
BPC = B // NCORES  # batches per core

MIN_DISTANCE = 2.9
LOSS_WEIGHT = 1.0
EPS_GUARD = 1e-4  # keeps dist_sq positive despite PSUM accumulation rounding

K_AUG = 18
P = 128
NRB = N // P  # row blocks per batch
ROW_TILING = False  # 4-way PE row-group tiling (tile_position)

# ---------------------------------------------------------------------------
# PSUM tile plan: 3 tiles x 1536 f32 (3 banks each, pool bufs=2 -> 6 banks).
# Chunks (row_block, col_start_local, width, col_start_global); each chunk
# stays within a 512-col PSUM bank segment.
# Tile 0 locals [0,1024) are the 8 diagonal blocks; everything else is
# strict-upper off-diagonal block columns.
TILES = [
    (1536, [(r, 128 * r, 128, 128 * r) for r in range(8)]
     + [(4, 1024, 384, 640), (6, 1408, 128, 896)]),
    (1536, [(2, 0, 512, 384), (2, 512, 128, 896),
            (0, 640, 384, 128), (0, 1024, 512, 512)]),
    (1536, [(3, 0, 512, 512), (1, 512, 512, 256),
            (1, 1024, 256, 768), (5, 1280, 256, 768)]),
]
D_W = sum(tw for tw, _ in TILES)  # 4608
D_OFF = np.cumsum([0] + [tw for tw, _ in TILES])[:-1]
DIAG_W = 1024  # diagonal-block region at d[:, 0:DIAG_W]

_cache = {}


def _build():
    if "nc" in _cache:
        return _cache["nc"]
    f32 = mybir.dt.float32
    bf16 = mybir.dt.bfloat16
    fp16 = mybir.dt.float16

    nc = bacc.Bacc("TRN2", target_bir_lowering=False, debug=False,
                   enable_asserts=False, num_devices=NCORES)
    if ROW_TILING:
        # lhs: [BPC, 128, 256] - row-block r's [18,128] lhsT slab lives at
        #   partitions 32*(r%4).. and columns 128*(r//4)..
        # rhs: [BPC, 128, 1024] - aug rhs replicated at bases 0/32/64/96
        lhs_d = nc.dram_tensor("lhs", [BPC, P, 2 * P], bf16,
                               kind="ExternalInput").ap()
        rhs_d = nc.dram_tensor("rhs", [BPC, P, N], bf16,
                               kind="ExternalInput").ap()
    else:
        lhs_d = nc.dram_tensor("lhs", [BPC, K_AUG, N], bf16,
                               kind="ExternalInput").ap()
        rhs_d = nc.dram_tensor("rhs", [BPC, K_AUG, N], bf16,
                               kind="ExternalInput").ap()
    stats_d = nc.dram_tensor("stats", [P, 2 * BPC], f32, kind="ExternalOutput").ap()

    with tile.TileContext(nc) as tc:
        with (
            tc.tile_pool(name="inp", bufs=6) as inp,
            tc.tile_pool(name="dpool", bufs=3) as dpool,
            tc.tile_pool(name="tpool", bufs=2) as tpool,
            tc.tile_pool(name="sqpool", bufs=2) as sqpool,
            tc.tile_pool(name="dmy", bufs=1) as dmy,
            tc.tile_pool(name="spool", bufs=1) as spool,
            tc.tile_pool(name="psum", bufs=2, space="PSUM") as psum,
        ):
            for b in range(BPC):
                if ROW_TILING:
                    lhs_sb = inp.tile([P, 2 * P], bf16, tag="lhs_t")
                    rhs_sb = inp.tile([P, N], bf16, tag="rhs_t")
                else:
                    lhs_sb = inp.tile([K_AUG, N], bf16, tag="lhs_t")
                    rhs_sb = inp.tile([K_AUG, N], bf16, tag="rhs_t")
                nc.sync.dma_start(out=lhs_sb, in_=lhs_d[b])
                nc.gpsimd.dma_start(out=rhs_sb, in_=rhs_d[b])

                d_sb = dpool.tile([P, D_W], bf16, tag="d")
                for g, (tile_w, chunks) in enumerate(TILES):
                    pt = psum.tile([P, tile_w], f32, tag="pt")
                    for r, cs, w, jg in chunks:
                        if ROW_TILING:
                            q = r % 4
                            nc.tensor.matmul(
                                pt[:, cs:cs + w],
                                lhs_sb[32 * q:32 * q + K_AUG,
                                       P * (r // 4):P * (r // 4) + P],
                                rhs_sb[32 * q:32 * q + K_AUG, jg:jg + w],
                                start=True,
                                stop=True,
                                tile_position=(32 * q, 0),
                            )
                        else:
                            nc.tensor.matmul(
                                pt[:, cs:cs + w],
                                lhs_sb[:, P * r:P * (r + 1)],
                                rhs_sb[:, jg:jg + w],
                                start=True,
                                stop=True,
                            )
                    nc.scalar.activation(
                        out=d_sb[:, int(D_OFF[g]):int(D_OFF[g]) + tile_w],
                        in_=pt[:, :],
                        func=mybir.ActivationFunctionType.Sqrt,
                        bias=0.0,  # EPS_GUARD is already inside dist_sq
                        scale=1.0,
                    )

                # t = min(d - 2.9, 0)   (== -relu(2.9 - d)), fp16, 4x mode
                t_sb = tpool.tile([P, D_W], fp16, tag="t")
                nc.vector.tensor_scalar(
                    out=t_sb,
                    in0=d_sb,
                    scalar1=float(MIN_DISTANCE),
                    scalar2=0.0,
                    op0=mybir.AluOpType.subtract,
                    op1=mybir.AluOpType.min,
                )
                # sq = t*t, fp16, 2x mode
                sq_sb = sqpool.tile([P, D_W], fp16, tag="sq")
                nc.vector.tensor_tensor(
                    out=sq_sb, in0=t_sb, in1=t_sb, op=mybir.AluOpType.mult,
                )
                # accumulate: diag-block region and off-diag separately (4x)
                st_b = spool.tile([P, 2], f32, tag=f"st{b}")
                dmy_d = dmy.tile([P, DIAG_W], fp16, tag="dmy_d")
                nc.vector.tensor_scalar(
                    out=dmy_d, in0=sq_sb[:, 0:DIAG_W],
                    scalar1=1.0, scalar2=0.0,
                    op0=mybir.AluOpType.mult,
                    op1=mybir.AluOpType.add,
                    accum_out=st_b[:, 0:1],
                )
                dmy_o = dmy.tile([P, D_W - DIAG_W], fp16, tag="dmy_o")
                nc.vector.tensor_scalar(
                    out=dmy_o, in0=sq_sb[:, DIAG_W:D_W],
                    scalar1=1.0, scalar2=0.0,
                    op0=mybir.AluOpType.mult,
                    op1=mybir.AluOpType.add,
                    accum_out=st_b[:, 1:2],
                )
                nc.sync.dma_start(out=stats_d[:, 2 * b:2 * b + 2], in_=st_b)

    nc.compile()
    _cache["nc"] = nc
    return nc


def _bf16_split(x, n):
    """Split fp64 array into n bf16 terms summing to ~x."""
    import ml_dtypes

    out = []
    rem = x.copy()
    for _ in range(n):
        h = rem.astype(ml_dtypes.bfloat16)
        out.append(h)
        rem = rem - h.astype(np.float64)
    return out


def _prep_aug(coords):
    """Host-side: build the augmented K=18 lhs/rhs encoding (bf16 hi/lo)."""
    import ml_dtypes

    bf = ml_dtypes.bfloat16
    atoms = coords.reshape(B, N, 3).astype(np.float64)
    at = atoms.transpose(0, 2, 1)  # [B, 3, N]
    ah = at.astype(bf)
    al = (at - ah.astype(np.float64)).astype(bf)
    a_eff = ah.astype(np.float64) + al.astype(np.float64)
    s_eff = (a_eff * a_eff).sum(axis=1)  # [B, N] exact squared norms

    si = _bf16_split(s_eff, 3)
    sj = _bf16_split(s_eff + EPS_GUARD, 3)

    lhs = np.zeros((B, K_AUG, N), bf)
    rhs = np.zeros((B, K_AUG, N), bf)
    lhs[:, 0], lhs[:, 1], lhs[:, 2] = si
    rhs[:, 0:3] = 1.0
    for c in range(3):
        k = 3 + 4 * c
        m2ah = (-2.0 * ah[:, c].astype(np.float64)).astype(bf)
        m2al = (-2.0 * al[:, c].astype(np.float64)).astype(bf)
        lhs[:, k + 0], rhs[:, k + 0] = m2ah, ah[:, c]
        lhs[:, k + 1], rhs[:, k + 1] = m2ah, al[:, c]
        lhs[:, k + 2], rhs[:, k + 2] = m2al, ah[:, c]
        lhs[:, k + 3], rhs[:, k + 3] = m2al, al[:, c]
    lhs[:, 15:18] = 1.0
    rhs[:, 15], rhs[:, 16], rhs[:, 17] = sj
    return lhs, rhs


def _prep_inputs(lhs, rhs):
    """Pack aug lhs/rhs into the device layout."""
    import ml_dtypes

    bf = ml_dtypes.bfloat16
    if ROW_TILING:
        lhs4 = np.zeros((B, P, 2 * P), bf)
        rhs4 = np.zeros((B, P, N), bf)
        for r in range(NRB):
            q = r % 4
            lhs4[:, 32 * q:32 * q + K_AUG, P * (r // 4):P * (r // 4) + P] = \
                lhs[:, :, P * r:P * (r + 1)]
        for q in range(4):
            rhs4[:, 32 * q:32 * q + K_AUG, :] = rhs
    else:
        lhs4, rhs4 = lhs, rhs

    in_maps = []
    for c in range(NCORES):
        in_maps.append({
            "lhs": np.ascontiguousarray(lhs4[c * BPC:(c + 1) * BPC]),
            "rhs": np.ascontiguousarray(rhs4[c * BPC:(c + 1) * BPC]),
        })
    return in_maps


def _diag_estimates(lhs, rhs):
    """Exactly emulate the kernel's value for each true-diagonal element
    (i,i): sequential fp32 accumulation of the 18 exact products, sqrt,
    bf16 rounding, t = fp16(min(d-2.9,0)), sq = fp16(t*t).  Returns [B]
    sums of the sq values."""
    import ml_dtypes

    lhs64 = lhs.astype(np.float64)  # [B, 18, N]
    rhs64 = rhs.astype(np.float64)
    prods = lhs64 * rhs64  # products for (i,i), exact in f32
    acc = np.zeros((B, N), np.float32)
    for k in range(K_AUG):
        acc = (acc + prods[:, k].astype(np.float32)).astype(np.float32)
    d = np.sqrt(acc).astype(ml_dtypes.bfloat16).astype(np.float32)
    t = np.minimum(d - np.float32(MIN_DISTANCE), 0.0).astype(np.float16)
    sq = (t.astype(np.float32) * t.astype(np.float32)).astype(np.float16)
    return sq.astype(np.float64).sum(axis=1)  # [B]


def _run(coordinates, trace=False, **trace_kwargs):
    coords = np.asarray(coordinates, dtype=np.float32)
    assert coords.shape == (B, 3 * N), coords.shape
    nc = _build()
    lhs, rhs = _prep_aug(coords)
    in_maps = _prep_inputs(lhs, rhs)
    res = run_bass_kernel_spmd(nc, in_maps, core_ids=list(range(NCORES)),
                               trace=trace, **trace_kwargs)
    diag_est = _diag_estimates(lhs, rhs)
    total = 0.0
    for c in range(NCORES):
        st = res.results[c]["stats"].astype(np.float64)
        for b in range(BPC):
            s_diag = st[:, 2 * b].sum()
            s_off = st[:, 2 * b + 1].sum()
            total += s_off + 0.5 * (s_diag - diag_est[c * BPC + b])
    loss = np.float32(LOSS_WEIGHT * total / B)
    return loss, res


def kernel(coordinates):
    loss, _ = _run(coordinates)
    return np.asarray(loss, dtype=np.float32)
